# revision 19
# baseline (speedup 1.0000x reference)
"""Trainium2 Bass kernel for BertWithAdaThresholdLocContextPooling head.

Data-parallel over batch: 32 batches -> 8 NeuronCores x 4 batches.
Per core, everything is gather-based: only 8 mention rows of
sequence_output (entity 0) and 16*12 attention rows per batch are read
via indirect DMA -- the 400MB attention tensor is never fully read.

Math per batch b (faithful to the reference, including the hs-in-both-
extractors detail):
  hs  = logsumexp_m seq[pos[b,0,m]]                       [768]
  A_e = mean_m attention[:, pos[b,e,m], :]                [12, 512]
  w   = sum_h A_0 * A_1;  rs = (w @ seq[b]) / (sum(w) + 12e-5)
  x_f = tanh(W_f @ [hs | rs | ner_f | 1])   f in {head, tail}
  logits = W_bil @ vec(outer-per-group(x_head, x_tail)) + b_bil

All matmuls run with the contraction dim on SBUF partitions; activations
are kept feature-on-partition ("transposed") end to end so no on-device
transposes are needed. The grouped outer product is built with two
constant 16x128 replication matmuls per 128-feature chunk.
"""

import os

import numpy as np

import concourse.bass as bass
import concourse.tile as tile
from concourse import bacc, mybir
from concourse.bass_utils import run_bass_kernel_spmd

# problem dims
B, H, C, D = 32, 12, 512, 768
M = 8
EMB, BLK = 768, 8
NCLS, NER = 97, 6
OFFSET = 1
NCORES = 8
BL = B // NCORES            # batches per core
KIN = 2 * D + NER + 1       # 1543: [hs | rs | ner | 1]
KCH = 13                    # ceil(KIN / 128)
KLAST = KIN - 12 * 128      # 7 rows in the last chunk
GRP = EMB // BLK            # 96 bilinear groups
KP = EMB * BLK              # 6144
NT = KP // 128              # 48 bilinear chunks
F32 = mybir.dt.float32
BF16 = mybir.dt.bfloat16
I32 = mybir.dt.int32

# f32 const block [128, _CF_NCOL]: attention-path selectors + bias
_CF_SEL96 = 0       # [96,12]  mention-mean selector (1/M)
_CF_ONES128 = 12    # [1,128]
_CF_BBIL = 140      # [97,1]   bilinear bias
_CF_NCOL = 141
# bf16 const block [128, _CB_NCOL]: mention-sum + head-sum selectors
_CB_SEL32 = 0       # [32,4]
_CB_ONES12 = 4      # [12,1]
_CB_NCOL = 5
# bf16 selector block [128, 512]: bilinear row replicators, [32,128] each,
# tiled at partition bases 0/32/64/96; cols = [A_v0 | A_v1 | B_v0 | B_v1]
_SAB_NCOL = 512

_CACHE = {}

LAST_EXEC_NS = None
LAST_RESULTS = None


def _build_nc():
    nc = bacc.Bacc("TRN2", target_bir_lowering=False, debug=False)

    seq_h = nc.dram_tensor("seq", [BL * C, D], BF16, kind="ExternalInput")
    attn_h = nc.dram_tensor("attn", [BL * H * C, C], F32, kind="ExternalInput")
    midx_h = nc.dram_tensor("midx", [BL * M, 1], I32, kind="ExternalInput")
    aidx_h = nc.dram_tensor("aidx", [M * H, 2 * BL], I32, kind="ExternalInput")
    nert_h = nc.dram_tensor("nert", [2 * (NER + 1), BL], BF16, kind="ExternalInput")
    wh_h = nc.dram_tensor("whT", [128, KCH * EMB], BF16, kind="ExternalInput")
    wt_h = nc.dram_tensor("wtT", [128, KCH * EMB], BF16, kind="ExternalInput")
    wb_h = nc.dram_tensor("wbT", [128, NT * NCLS], BF16, kind="ExternalInput")
    cstf_h = nc.dram_tensor("cstf", [128, _CF_NCOL], F32, kind="ExternalInput")
    cstb_h = nc.dram_tensor("cstb", [128, _CB_NCOL], BF16, kind="ExternalInput")
    sab_h = nc.dram_tensor("sab", [128, _SAB_NCOL], BF16, kind="ExternalInput")
    out_h = nc.dram_tensor("outT", [NCLS, BL], F32, kind="ExternalOutput")

    AF = mybir.ActivationFunctionType
    OP = mybir.AluOpType

    with tile.TileContext(nc) as tc:
        with (
            tc.tile_pool(name="w", bufs=1) as wp,
            tc.tile_pool(name="seqp", bufs=3) as sp,
            tc.tile_pool(name="g", bufs=2) as gp,
            tc.tile_pool(name="blp", bufs=3) as blp,
            tc.tile_pool(name="ps", bufs=8, space="PSUM") as pp,
        ):
            # small loads first on the scalar HWDGE ring; big weights on the
            # sync ring so they don't head-of-line-block the small/seq loads.
            midx_sb = wp.tile([BL * M, 1], I32)
            nc.scalar.dma_start(out=midx_sb[:], in_=midx_h[:])
            aidx_sb = wp.tile([M * H, 2 * BL], I32)
            nc.scalar.dma_start(out=aidx_sb[:], in_=aidx_h[:])
            cstf_sb = wp.tile([128, _CF_NCOL], F32)
            nc.scalar.dma_start(out=cstf_sb[:], in_=cstf_h[:])
            cstb_sb = wp.tile([128, _CB_NCOL], BF16)
            nc.scalar.dma_start(out=cstb_sb[:], in_=cstb_h[:])
            sab_sb = wp.tile([128, _SAB_NCOL], BF16)
            nc.scalar.dma_start(out=sab_sb[:], in_=sab_h[:])
            # two tiles so both matmul rhs operands sit at partition base 0
            nerh_sb = wp.tile([NER + 1, BL], BF16)
            nc.scalar.dma_start(out=nerh_sb[:], in_=nert_h[0 : NER + 1, :])
            nertl_sb = wp.tile([NER + 1, BL], BF16)
            nc.scalar.dma_start(out=nertl_sb[:], in_=nert_h[NER + 1 : 2 * (NER + 1), :])

            wh_sb = wp.tile([128, KCH * EMB], BF16)
            nc.sync.dma_start(out=wh_sb[:], in_=wh_h[:])
            wt_sb = wp.tile([128, KCH * EMB], BF16)
            nc.sync.dma_start(out=wt_sb[:], in_=wt_h[:])
            wb_sb = wp.tile([128, NT * NCLS], BF16)
            nc.sync.dma_start(out=wb_sb[:], in_=wb_h[:])

            sel96 = cstf_sb[0 : M * H, _CF_SEL96 : _CF_SEL96 + H]
            ones128 = cstf_sb[0:1, _CF_ONES128 : _CF_ONES128 + 128]
            bbil = cstf_sb[0:NCLS, _CF_BBIL : _CF_BBIL + 1]
            sel32 = cstb_sb[0 : BL * M, _CB_SEL32 : _CB_SEL32 + BL]
            ones12 = cstb_sb[0:H, _CB_ONES12 : _CB_ONES12 + 1]

            # inpT[:, c, :] = chunk c of [hs | rs] with features on partitions
            inpT = wp.tile([128, 12, BL], BF16)

            # ---- phase 1: mention gather + logsumexp -> inpT chunks 0..5
            g_ment = gp.tile([BL * M, D], BF16)
            nc.gpsimd.indirect_dma_start(
                out=g_ment[:],
                out_offset=None,
                in_=seq_h[:],
                in_offset=bass.IndirectOffsetOnAxis(ap=midx_sb[:, 0:1], axis=0),
            )
            expm = gp.tile([BL * M, D], BF16)
            nc.scalar.activation(expm[:], g_ment[:], AF.Exp)
            for c in range(6):
                lse_ps = pp.tile([128, BL], F32, tag="ps", name="lse_ps")
                nc.tensor.matmul(
                    out=lse_ps[:],
                    lhsT=expm[:, c * 128 : (c + 1) * 128],
                    rhs=sel32,
                    start=True,
                    stop=True,
                )
                nc.scalar.activation(inpT[:, c, :], lse_ps[:], AF.Ln)

            # ---- phase 2: attention gathers -> normalized context weights
            # wTn_sb[:, 4*b + c] = ht_att[b, c*128 + p] (c = seq chunk)
            wTn_sb = wp.tile([128, BL * 4], BF16)
            sraw = wp.tile([1, BL], F32)
            sden = wp.tile([1, BL], F32)
            srec = wp.tile([1, BL], F32)
            for b in range(BL):
                P_ps = []
                for e in range(2):
                    att_g = gp.tile([M * H, C], F32, tag="attg", name=f"att_g{e}")
                    col = b * 2 + e
                    nc.gpsimd.indirect_dma_start(
                        out=att_g[:],
                        out_offset=None,
                        in_=attn_h[:],
                        in_offset=bass.IndirectOffsetOnAxis(
                            ap=aidx_sb[:, col : col + 1], axis=0
                        ),
                    )
                    p_ps = pp.tile([H, C], F32, tag="ps", name=f"p_ps{e}")
                    nc.tensor.matmul(
                        out=p_ps[:], lhsT=sel96, rhs=att_g[:], start=True, stop=True
                    )
                    P_ps.append(p_ps)
                p0_sb = gp.tile([H, C], F32, tag="p0sb")
                nc.scalar.activation(p0_sb[:], P_ps[0][:], AF.Copy)
                prod = gp.tile([H, C], BF16, tag="prod")
                nc.vector.tensor_tensor(
                    out=prod[:], in0=p0_sb[:], in1=P_ps[1][:], op=OP.mult
                )
                w_ps = pp.tile([1, C], F32, tag="ps", name="w_ps")
                nc.tensor.matmul(
                    out=w_ps[:], lhsT=ones12, rhs=prod[:], start=True, stop=True
                )
                nc.vector.reduce_sum(
                    out=sraw[0:1, b : b + 1], in_=w_ps[:], axis=mybir.AxisListType.X
                )
                wT_ps = pp.tile([128, 4], F32, tag="ps", name="wT_ps")
                for c in range(4):
                    nc.tensor.matmul(
                        out=wT_ps[:, c : c + 1],
                        lhsT=prod[:, c * 128 : (c + 1) * 128],
                        rhs=ones12,
                        start=True,
                        stop=True,
                    )
                # ht = w / (sum(w) + H*1e-5); denominator folds the /H and /M^2
                nc.vector.tensor_scalar_add(
                    out=sden[0:1, b : b + 1],
                    in0=sraw[0:1, b : b + 1],
                    scalar1=float(H) * 1e-5,
                )
                nc.vector.reciprocal(out=srec[0:1, b : b + 1], in_=sden[0:1, b : b + 1])
                sb_ps = pp.tile([128, 1], F32, tag="ps", name="sb_ps")
                nc.tensor.matmul(
                    out=sb_ps[:],
                    lhsT=ones128,
                    rhs=srec[0:1, b : b + 1],
                    start=True,
                    stop=True,
                )
                sb_sb = blp.tile([128, 1], F32, tag="sb_sb")
                nc.vector.tensor_copy(sb_sb[:], sb_ps[:])
                nc.vector.tensor_tensor(
                    out=wTn_sb[:, b * 4 : (b + 1) * 4],
                    in0=wT_ps[:],
                    in1=sb_sb[:, 0:1].to_broadcast([128, 4]),
                    op=OP.mult,
                )  # wTn_sb is bf16; DVE rounds on write

            # ---- phase 3: rs = ht_att @ seq  (streamed seq tiles)
            rsT_ps = [
                pp.tile([128, BL], F32, tag="ps", name=f"rsT_ps{j}") for j in range(6)
            ]
            for b in range(BL):
                for c in range(4):
                    seq_t = sp.tile([128, D], BF16, tag="seqt")
                    r0 = (b * 4 + c) * 128
                    nc.scalar.dma_start(out=seq_t[:], in_=seq_h[r0 : r0 + 128, :])
                    for j in range(6):
                        nc.tensor.matmul(
                            out=rsT_ps[j][:, b : b + 1],
                            lhsT=seq_t[:, j * 128 : (j + 1) * 128],
                            rhs=wTn_sb[:, (b * 4 + c) : (b * 4 + c) + 1],
                            start=(c == 0),
                            stop=(c == 3),
                        )
            for j in range(6):
                nc.vector.tensor_copy(inpT[:, 6 + j, :], rsT_ps[j][:])

            # ---- phase 4: extractors (features on partitions)
            # ex_fT[:, j, 0:4] = tanh head feats [j*128, (j+1)*128) for 4 batches
            # ex_fT[:, j, 4:8] = tanh tail feats
            ex_fT = wp.tile([128, 6, 2 * BL], BF16)
            for j in range(6):
                exh_ps = pp.tile([128, BL], F32, tag="ps", name="exh_ps")
                ext_ps = pp.tile([128, BL], F32, tag="ps", name="ext_ps")
                for c in range(KCH):
                    if c < 12:
                        lh = wh_sb[:, c * EMB + j * 128 : c * EMB + (j + 1) * 128]
                        lt = wt_sb[:, c * EMB + j * 128 : c * EMB + (j + 1) * 128]
                        rh = inpT[:, c, :]
                        rt = inpT[:, c, :]
                    else:
                        lh = wh_sb[0:KLAST, c * EMB + j * 128 : c * EMB + (j + 1) * 128]
                        lt = wt_sb[0:KLAST, c * EMB + j * 128 : c * EMB + (j + 1) * 128]
                        rh = nerh_sb[:]
                        rt = nertl_sb[:]
                    nc.tensor.matmul(
                        out=exh_ps[:], lhsT=lh, rhs=rh, start=(c == 0), stop=(c == KCH - 1)
                    )
                    nc.tensor.matmul(
                        out=ext_ps[:], lhsT=lt, rhs=rt, start=(c == 0), stop=(c == KCH - 1)
                    )
                nc.scalar.activation(ex_fT[:, j, 0:BL], exh_ps[:], AF.Tanh)
                nc.scalar.activation(ex_fT[:, j, BL : 2 * BL], ext_ps[:], AF.Tanh)

            # ---- phase 5: grouped bilinear + output matmul
            logit_ps = pp.tile([NCLS, BL], F32, tag="ps", name="logit_ps")
            for t in range(NT):
                j6, r = t // 8, t % 8
                base, v = 32 * (r // 2), r % 2
                src = ex_fT[base : base + 32, j6, :]
                selA = sab_sb[base : base + 32, v * 128 : (v + 1) * 128]
                selB = sab_sb[base : base + 32, (2 + v) * 128 : (3 + v) * 128]
                psA = pp.tile([128, 2 * BL], F32, tag="ps", name="psA")
                psB = pp.tile([128, 2 * BL], F32, tag="ps", name="psB")
                nc.tensor.matmul(
                    out=psA[:], lhsT=selA, rhs=src, start=True, stop=True,
                    tile_position=(base, 0),
                )
                nc.tensor.matmul(
                    out=psB[:], lhsT=selB, rhs=src, start=True, stop=True,
                    tile_position=(base, 0),
                )
                psA_sb = blp.tile([128, BL], F32, tag="psA_sb")
                nc.scalar.activation(psA_sb[:], psA[:, 0:BL], AF.Copy)
                blT = blp.tile([128, BL], BF16, tag="blT")
                nc.vector.tensor_tensor(
                    out=blT[:], in0=psA_sb[:], in1=psB[:, BL : 2 * BL], op=OP.mult
                )
                nc.tensor.matmul(
                    out=logit_ps[:],
                    lhsT=wb_sb[:, t * NCLS : (t + 1) * NCLS],
                    rhs=blT[:],
                    start=(t == 0),
                    stop=(t == NT - 1),
                )
            logitsT_sb = wp.tile([NCLS, BL], F32)
            nc.vector.tensor_scalar_add(out=logitsT_sb[:], in0=logit_ps[:], scalar1=bbil)
            nc.scalar.dma_start(out=out_h[:], in_=logitsT_sb[:])

    nc.compile()
    return nc


def _bf16(x):
    import ml_dtypes

    return np.ascontiguousarray(np.asarray(x).astype(ml_dtypes.bfloat16))


def _weights_prep(W_head, b_head, W_tail, b_tail, W_bil, b_bil):
    """Host-side weight packing (transposed + chunk-interleaved + bias rows)."""

    def pack_ext(Wf, bf):
        ext = np.zeros((KCH * 128, EMB), np.float32)
        ext[: 2 * D + NER] = Wf.T.astype(np.float32)
        ext[2 * D + NER] = bf.astype(np.float32)
        return _bf16(
            ext.reshape(KCH, 128, EMB).transpose(1, 0, 2).reshape(128, KCH * EMB)
        )

    whT = pack_ext(W_head, b_head)
    wtT = pack_ext(W_tail, b_tail)

    wbe = np.asarray(W_bil, np.float32).T  # [KP, NCLS]
    wbT = _bf16(wbe.reshape(NT, 128, NCLS).transpose(1, 0, 2).reshape(128, NT * NCLS))

    cstf = np.zeros((128, _CF_NCOL), np.float32)
    for m in range(M):
        for h in range(H):
            cstf[m * H + h, _CF_SEL96 + h] = 1.0 / M
    cstf[0, _CF_ONES128 : _CF_ONES128 + 128] = 1.0
    cstf[0:NCLS, _CF_BBIL] = b_bil.astype(np.float32)

    cstb = np.zeros((128, _CB_NCOL), np.float32)
    for b in range(BL):
        for m in range(M):
            cstb[b * M + m, _CB_SEL32 + b] = 1.0
    cstb[0:H, _CB_ONES12] = 1.0

    # [32,128] replicators, variant v covers bl-chunk rows 16v..16v+15,
    # tiled at every 32-partition base so lhsT/rhs partition bases match
    sab32 = np.zeros((32, _SAB_NCOL), np.float32)
    p = np.arange(128)
    srcA = (p // 64) * 8 + (p % 64) // 8
    srcB = (p // 64) * 8 + (p % 8)
    for v in range(2):
        sab32[16 * v + srcA, v * 128 + p] = 1.0
        sab32[16 * v + srcB, (2 + v) * 128 + p] = 1.0
    sab = np.tile(sab32, (4, 1))
    return whT, wtT, wbT, cstf, _bf16(cstb), _bf16(sab)


def _make_in_maps(inputs):
    seq = np.ascontiguousarray(np.asarray(inputs["sequence_output"], np.float32))
    att = np.ascontiguousarray(np.asarray(inputs["attention"], np.float32))
    ner = np.asarray(inputs["ner_tags"], np.float32)
    ep = np.asarray(inputs["entity_pos"]).astype(np.int64)
    pos = ep + OFFSET  # [B, 2, M]

    whT, wtT, wbT, cstf, cstb, sab = _weights_prep(
        np.asarray(inputs["W_head"]),
        np.asarray(inputs["b_head"]),
        np.asarray(inputs["W_tail"]),
        np.asarray(inputs["b_tail"]),
        np.asarray(inputs["W_bil"]),
        np.asarray(inputs["b_bil"]),
    )

    in_maps = []
    for k in range(NCORES):
        b0 = k * BL
        seq_k = _bf16(seq[b0 : b0 + BL].reshape(BL * C, D))
        att_k = att[b0 : b0 + BL].reshape(BL * H * C, C)

        midx = np.zeros((BL * M, 1), np.int32)
        for b in range(BL):
            midx[b * M : (b + 1) * M, 0] = b * C + pos[b0 + b, 0, :]

        aidx = np.zeros((M * H, 2 * BL), np.int32)
        mh_h = np.tile(np.arange(H), M)  # row p = m*H + h -> h
        mh_m = np.repeat(np.arange(M), H)  # -> m
        for b in range(BL):
            for e in range(2):
                aidx[:, b * 2 + e] = (b * H + mh_h) * C + pos[b0 + b, e, mh_m]

        nert = np.zeros((2 * (NER + 1), BL), np.float32)
        nert[0:NER] = ner[b0 : b0 + BL, 0, :].T
        nert[NER] = 1.0
        nert[NER + 1 : 2 * NER + 1] = ner[b0 : b0 + BL, 1, :].T
        nert[2 * NER + 1] = 1.0

        in_maps.append(
            {
                "seq": seq_k,
                "attn": np.ascontiguousarray(att_k),
                "midx": midx,
                "aidx": aidx,
                "nert": _bf16(nert),
                "whT": whT,
                "wtT": wtT,
                "wbT": wbT,
                "cstf": cstf,
                "cstb": cstb,
                "sab": sab,
            }
        )
    return in_maps


def _get_nc():
    if "nc" not in _CACHE:
        _CACHE["nc"] = _build_nc()
    return _CACHE["nc"]


def kernel(**inputs):
    global LAST_EXEC_NS, LAST_RESULTS
    nc = _get_nc()
    in_maps = _make_in_maps(inputs)
    trace = bool(int(os.environ.get("BASS_KERNEL_TRACE", "0")))
    try:
        res = run_bass_kernel_spmd(
            nc, in_maps, core_ids=list(range(NCORES)), trace=trace
        )
    except Exception:
        if not trace:
            raise
        # tracing infra unavailable in this environment -- run untraced
        res = run_bass_kernel_spmd(
            nc, in_maps, core_ids=list(range(NCORES)), trace=False
        )
    LAST_EXEC_NS = res.exec_time_ns
    LAST_RESULTS = res
    out = np.zeros((B, NCLS), np.float32)
    for k in range(NCORES):
        out[k * BL : (k + 1) * BL] = np.asarray(res.results[k]["outT"]).T
    return out


# revision 26
# speedup vs baseline: 1.6806x; 1.6806x over previous
"""Trainium2 Bass kernel for BertWithAdaThresholdLocContextPooling head.

Data-parallel over batch: 32 batches -> 8 NeuronCores x 4 batches.
Per core, everything is gather-based: only 8 mention rows of
sequence_output (entity 0) and 16*12 attention rows per batch are read
via indirect DMA -- the 400MB attention tensor is never fully read.

Math per batch b (faithful to the reference, including the hs-in-both-
extractors detail):
  hs  = logsumexp_m seq[pos[b,0,m]]                       [768]
  A_e = mean_m attention[:, pos[b,e,m], :]                [12, 512]
  w   = sum_h A_0 * A_1;  rs = (w @ seq[b]) / (sum(w) + 12e-5)
  x_f = tanh(W_f @ [hs | rs | ner_f | 1])   f in {head, tail}
  logits = W_bil @ vec(outer-per-group(x_head, x_tail)) + b_bil

All matmuls run with the contraction dim on SBUF partitions; activations
are kept feature-on-partition ("transposed") end to end so no on-device
transposes are needed. The grouped outer product is built with two
constant 16x128 replication matmuls per 128-feature chunk.
"""

import os

import numpy as np

import concourse.bass as bass
import concourse.tile as tile
from concourse import bacc, mybir
from concourse.bass_utils import run_bass_kernel_spmd

# problem dims
B, H, C, D = 32, 12, 512, 768
M = 8
EMB, BLK = 768, 8
NCLS, NER = 97, 6
OFFSET = 1
NCORES = 8
BL = B // NCORES            # batches per core
KIN = 2 * D + NER + 1       # 1543: [hs | rs | ner | 1]
KCH = 13                    # ceil(KIN / 128)
KLAST = KIN - 12 * 128      # 7 rows in the last chunk
GRP = EMB // BLK            # 96 bilinear groups
KP = EMB * BLK              # 6144
NT = KP // 128              # 48 bilinear chunks
F32 = mybir.dt.float32
BF16 = mybir.dt.bfloat16
I32 = mybir.dt.int32

# f32 const block [128, _CF_NCOL]: attention-path selectors + bias
_CF_SEL96 = 0       # [96,12]  mention-mean selector (1/M)
_CF_ONES128 = 12    # [1,128]
_CF_BBIL = 140      # [97,1]   bilinear bias
_CF_NCOL = 141
# bf16 const block [128, _CB_NCOL]: mention-sum + head-sum selectors
_CB_SEL32 = 0       # [32,4]
_CB_ONES12 = 4      # [12,1]
_CB_ONES128C = 5    # [128,1]
_CB_NCOL = 6
# bf16 selector block [128, 512]: bilinear row replicators, [32,128] each,
# tiled at partition bases 0/32/64/96; cols = [A_v0 | A_v1 | B_v0 | B_v1]
_SAB_NCOL = 512

_CACHE = {}

LAST_EXEC_NS = None
LAST_RESULTS = None


def _build_nc():
    nc = bacc.Bacc("TRN2", target_bir_lowering=False, debug=False)

    seq_h = nc.dram_tensor("seq", [BL * C, D], BF16, kind="ExternalInput")
    attn_h = nc.dram_tensor("attn", [BL * H * C, C], F32, kind="ExternalInput")
    midx_h = nc.dram_tensor("midx", [BL * M, 1], I32, kind="ExternalInput")
    aidx_h = nc.dram_tensor("aidx", [M * H, 2 * BL], I32, kind="ExternalInput")
    nert_h = nc.dram_tensor("nert", [2 * (NER + 1), BL], BF16, kind="ExternalInput")
    wh_h = nc.dram_tensor("whT", [128, KCH * EMB], BF16, kind="ExternalInput")
    wt_h = nc.dram_tensor("wtT", [128, KCH * EMB], BF16, kind="ExternalInput")
    wb_h = nc.dram_tensor("wbT", [128, NT * NCLS], BF16, kind="ExternalInput")
    cstf_h = nc.dram_tensor("cstf", [128, _CF_NCOL], F32, kind="ExternalInput")
    cstb_h = nc.dram_tensor("cstb", [128, _CB_NCOL], BF16, kind="ExternalInput")
    sab_h = nc.dram_tensor("sab", [128, _SAB_NCOL], BF16, kind="ExternalInput")
    out_h = nc.dram_tensor("outT", [NCLS, BL], F32, kind="ExternalOutput")

    AF = mybir.ActivationFunctionType
    OP = mybir.AluOpType

    with tile.TileContext(nc) as tc:
        with (
            tc.tile_pool(name="w", bufs=1) as wp,
            tc.tile_pool(name="seqp", bufs=1) as sp,
            tc.tile_pool(name="g", bufs=2) as gp,
            tc.tile_pool(name="blp", bufs=3) as blp,
            tc.tile_pool(name="ps", bufs=8, space="PSUM") as pp,
        ):
            # small loads first on the scalar HWDGE ring; big weights on the
            # sync ring so they don't head-of-line-block the small/seq loads.
            midx_sb = wp.tile([BL * M, 1], I32)
            nc.scalar.dma_start(out=midx_sb[:], in_=midx_h[:])
            aidx_sb = wp.tile([M * H, 2 * BL], I32)
            nc.scalar.dma_start(out=aidx_sb[:], in_=aidx_h[:])
            cstf_sb = wp.tile([128, _CF_NCOL], F32)
            nc.scalar.dma_start(out=cstf_sb[:], in_=cstf_h[:])
            cstb_sb = wp.tile([128, _CB_NCOL], BF16)
            nc.scalar.dma_start(out=cstb_sb[:], in_=cstb_h[:])
            sab_sb = wp.tile([128, _SAB_NCOL], BF16)
            nc.scalar.dma_start(out=sab_sb[:], in_=sab_h[:])
            # two tiles so both matmul rhs operands sit at partition base 0
            nerh_sb = wp.tile([NER + 1, BL], BF16)
            nc.scalar.dma_start(out=nerh_sb[:], in_=nert_h[0 : NER + 1, :])
            nertl_sb = wp.tile([NER + 1, BL], BF16)
            nc.scalar.dma_start(out=nertl_sb[:], in_=nert_h[NER + 1 : 2 * (NER + 1), :])

            # big loads spread across the three DMA-capable queues: seq split
            # over SP+ACT (needed first), then whT/wbT on SP; wtT rides the
            # Pool queue behind the gathers (only the tail extractor needs it).
            seq_ts = []
            for i in range(4 * BL):
                seq_t = sp.tile([128, D], BF16, name=f"seq_t{i}")
                eng = nc.sync if i % 2 == 0 else nc.scalar
                eng.dma_start(out=seq_t[:], in_=seq_h[i * 128 : (i + 1) * 128, :])
                seq_ts.append(seq_t)
            wh_sb = wp.tile([128, KCH * EMB], BF16)
            nc.sync.dma_start(out=wh_sb[:], in_=wh_h[:])
            wb_sb = wp.tile([128, NT * NCLS], BF16)
            nc.sync.dma_start(out=wb_sb[:], in_=wb_h[:])

            sel96 = cstf_sb[0 : M * H, _CF_SEL96 : _CF_SEL96 + H]
            ones128 = cstf_sb[0:1, _CF_ONES128 : _CF_ONES128 + 128]
            bbil = cstf_sb[0:NCLS, _CF_BBIL : _CF_BBIL + 1]
            sel32 = cstb_sb[0 : BL * M, _CB_SEL32 : _CB_SEL32 + BL]
            ones12 = cstb_sb[0:H, _CB_ONES12 : _CB_ONES12 + 1]
            onescol = cstb_sb[0:128, _CB_ONES128C : _CB_ONES128C + 1]

            # inpT[:, c, :] = chunk c of [hs | rs] with features on partitions
            inpT = wp.tile([128, 12, BL], BF16)

            # ---- phase 1: mention gather + logsumexp -> inpT chunks 0..5
            g_ment = gp.tile([BL * M, D], BF16)
            nc.gpsimd.indirect_dma_start(
                out=g_ment[:],
                out_offset=None,
                in_=seq_h[:],
                in_offset=bass.IndirectOffsetOnAxis(ap=midx_sb[:, 0:1], axis=0),
            )
            expm = gp.tile([BL * M, D], BF16)
            nc.scalar.activation(expm[:], g_ment[:], AF.Exp)
            for c in range(6):
                lse_ps = pp.tile([128, BL], F32, tag="ps", name="lse_ps")
                nc.tensor.matmul(
                    out=lse_ps[:],
                    lhsT=expm[:, c * 128 : (c + 1) * 128],
                    rhs=sel32,
                    start=True,
                    stop=True,
                )
                nc.scalar.activation(inpT[:, c, :], lse_ps[:], AF.Ln)

            # ---- phase 2: attention gathers -> normalized context weights
            # wTn_sb[:, 4*b + c] = ht_att[b, c*128 + p] (c = seq chunk)
            wTn_sb = wp.tile([128, BL * 4], BF16)
            sraw = wp.tile([1, BL], F32)
            sden = wp.tile([1, BL], F32)
            srec = wp.tile([1, BL], F32)
            for b in range(BL):
                P_ps = []
                for e in range(2):
                    att_g = gp.tile([M * H, C], F32, tag="attg", name=f"att_g{e}")
                    col = b * 2 + e
                    nc.gpsimd.indirect_dma_start(
                        out=att_g[:],
                        out_offset=None,
                        in_=attn_h[:],
                        in_offset=bass.IndirectOffsetOnAxis(
                            ap=aidx_sb[:, col : col + 1], axis=0
                        ),
                    )
                    p_ps = pp.tile([H, C], F32, tag="ps", name=f"p_ps{e}")
                    nc.tensor.matmul(
                        out=p_ps[:], lhsT=sel96, rhs=att_g[:], start=True, stop=True
                    )
                    P_ps.append(p_ps)
                prod = gp.tile([H, C], BF16, tag="prod")
                nc.vector.tensor_tensor(
                    out=prod[:], in0=P_ps[0][:], in1=P_ps[1][:], op=OP.mult
                )
                wT_ps = pp.tile([128, 4], F32, tag="ps", name="wT_ps")
                for c in range(4):
                    nc.tensor.matmul(
                        out=wT_ps[:, c : c + 1],
                        lhsT=prod[:, c * 128 : (c + 1) * 128],
                        rhs=ones12,
                        start=True,
                        stop=True,
                    )
                wT_sb = blp.tile([128, 4], BF16, tag="wT_sb")
                nc.vector.tensor_copy(wT_sb[:], wT_ps[:])
                # s_b = sum(w) via PE (column sums then a 4-wide reduce)
                s_ps = pp.tile([1, 4], F32, tag="ps", name="s_ps")
                nc.tensor.matmul(
                    out=s_ps[:], lhsT=onescol, rhs=wT_sb[:], start=True, stop=True
                )
                nc.vector.reduce_sum(
                    out=sraw[0:1, b : b + 1], in_=s_ps[:], axis=mybir.AxisListType.X
                )
                # ht = w / (sum(w) + H*1e-5); denominator folds the /H and /M^2
                nc.vector.tensor_scalar_add(
                    out=sden[0:1, b : b + 1],
                    in0=sraw[0:1, b : b + 1],
                    scalar1=float(H) * 1e-5,
                )
                nc.vector.reciprocal(out=srec[0:1, b : b + 1], in_=sden[0:1, b : b + 1])
                sb_ps = pp.tile([128, 1], F32, tag="ps", name="sb_ps")
                nc.tensor.matmul(
                    out=sb_ps[:],
                    lhsT=ones128,
                    rhs=srec[0:1, b : b + 1],
                    start=True,
                    stop=True,
                )
                nc.vector.tensor_tensor(
                    out=wTn_sb[:, b * 4 : (b + 1) * 4],
                    in0=wT_sb[:],
                    in1=sb_ps[:, 0:1].to_broadcast([128, 4]),
                    op=OP.mult,
                )  # wTn_sb is bf16; DVE rounds on write

            # wtT lands on the Pool queue behind the gathers; needed only by
            # the tail extractor, well after this point.
            wt_sb = wp.tile([128, KCH * EMB], BF16)
            nc.gpsimd.dma_start(out=wt_sb[:], in_=wt_h[:])

            # ---- phase 3: rs = ht_att @ seq  (resident seq tiles)
            rsT_ps = [
                pp.tile([128, BL], F32, tag="ps", name=f"rsT_ps{j}") for j in range(6)
            ]
            for b in range(BL):
                for c in range(4):
                    seq_t = seq_ts[b * 4 + c]
                    for j in range(6):
                        nc.tensor.matmul(
                            out=rsT_ps[j][:, b : b + 1],
                            lhsT=seq_t[:, j * 128 : (j + 1) * 128],
                            rhs=wTn_sb[:, (b * 4 + c) : (b * 4 + c) + 1],
                            start=(c == 0),
                            stop=(c == 3),
                        )
            for j in range(6):
                nc.vector.tensor_copy(inpT[:, 6 + j, :], rsT_ps[j][:])

            # ---- phase 4: extractors (features on partitions)
            # ex_fT[:, j, 0:4] = tanh head feats [j*128, (j+1)*128) for 4 batches
            # ex_fT[:, j, 4:8] = tanh tail feats
            ex_fT = wp.tile([128, 6, 2 * BL], BF16)
            for j in range(6):
                exh_ps = pp.tile([128, BL], F32, tag="ps", name="exh_ps")
                ext_ps = pp.tile([128, BL], F32, tag="ps", name="ext_ps")
                for c in range(KCH):
                    if c < 12:
                        lh = wh_sb[:, c * EMB + j * 128 : c * EMB + (j + 1) * 128]
                        lt = wt_sb[:, c * EMB + j * 128 : c * EMB + (j + 1) * 128]
                        rh = inpT[:, c, :]
                        rt = inpT[:, c, :]
                    else:
                        lh = wh_sb[0:KLAST, c * EMB + j * 128 : c * EMB + (j + 1) * 128]
                        lt = wt_sb[0:KLAST, c * EMB + j * 128 : c * EMB + (j + 1) * 128]
                        rh = nerh_sb[:]
                        rt = nertl_sb[:]
                    nc.tensor.matmul(
                        out=exh_ps[:], lhsT=lh, rhs=rh, start=(c == 0), stop=(c == KCH - 1)
                    )
                    nc.tensor.matmul(
                        out=ext_ps[:], lhsT=lt, rhs=rt, start=(c == 0), stop=(c == KCH - 1)
                    )
                nc.scalar.activation(ex_fT[:, j, 0:BL], exh_ps[:], AF.Tanh)
                nc.scalar.activation(ex_fT[:, j, BL : 2 * BL], ext_ps[:], AF.Tanh)

            # ---- phase 5: grouped bilinear + output matmul
            logit_ps = pp.tile([NCLS, BL], F32, tag="ps", name="logit_ps")
            for tg in range(NT // 4):
                psA4 = pp.tile([128, 4 * 2 * BL], F32, tag="ps", name="psA4")
                psB4 = pp.tile([128, 4 * 2 * BL], F32, tag="ps", name="psB4")
                for i in range(4):
                    t = tg * 4 + i
                    j6, r = t // 8, t % 8
                    base, v = 32 * (r // 2), r % 2
                    src = ex_fT[base : base + 32, j6, :]
                    selA = sab_sb[base : base + 32, v * 128 : (v + 1) * 128]
                    selB = sab_sb[base : base + 32, (2 + v) * 128 : (3 + v) * 128]
                    nc.tensor.matmul(
                        out=psA4[:, i * 8 : (i + 1) * 8], lhsT=selA, rhs=src,
                        start=True, stop=True, tile_position=(base, 0),
                    )
                    nc.tensor.matmul(
                        out=psB4[:, i * 8 : (i + 1) * 8], lhsT=selB, rhs=src,
                        start=True, stop=True, tile_position=(base, 0),
                    )
                blT4 = blp.tile([128, 4, BL], BF16, tag="blT4")
                nc.vector.tensor_tensor(
                    out=blT4[:, :, :],
                    in0=psA4[:].rearrange("p (i c) -> p i c", c=8)[:, :, 0:BL],
                    in1=psB4[:].rearrange("p (i c) -> p i c", c=8)[:, :, BL : 2 * BL],
                    op=OP.mult,
                )
                for i in range(4):
                    t = tg * 4 + i
                    nc.tensor.matmul(
                        out=logit_ps[:],
                        lhsT=wb_sb[:, t * NCLS : (t + 1) * NCLS],
                        rhs=blT4[:, i, :],
                        start=(t == 0),
                        stop=(t == NT - 1),
                    )
            logitsT_sb = wp.tile([NCLS, BL], F32)
            nc.vector.tensor_scalar_add(out=logitsT_sb[:], in0=logit_ps[:], scalar1=bbil)
            nc.scalar.dma_start(out=out_h[:], in_=logitsT_sb[:])

    nc.compile()
    return nc


def _bf16(x):
    import ml_dtypes

    return np.ascontiguousarray(np.asarray(x).astype(ml_dtypes.bfloat16))


def _weights_prep(W_head, b_head, W_tail, b_tail, W_bil, b_bil):
    """Host-side weight packing (transposed + chunk-interleaved + bias rows)."""

    def pack_ext(Wf, bf):
        ext = np.zeros((KCH * 128, EMB), np.float32)
        ext[: 2 * D + NER] = Wf.T.astype(np.float32)
        ext[2 * D + NER] = bf.astype(np.float32)
        return _bf16(
            ext.reshape(KCH, 128, EMB).transpose(1, 0, 2).reshape(128, KCH * EMB)
        )

    whT = pack_ext(W_head, b_head)
    wtT = pack_ext(W_tail, b_tail)

    wbe = np.asarray(W_bil, np.float32).T  # [KP, NCLS]
    wbT = _bf16(wbe.reshape(NT, 128, NCLS).transpose(1, 0, 2).reshape(128, NT * NCLS))

    cstf = np.zeros((128, _CF_NCOL), np.float32)
    for m in range(M):
        for h in range(H):
            cstf[m * H + h, _CF_SEL96 + h] = 1.0 / M
    cstf[0, _CF_ONES128 : _CF_ONES128 + 128] = 1.0
    cstf[0:NCLS, _CF_BBIL] = b_bil.astype(np.float32)

    cstb = np.zeros((128, _CB_NCOL), np.float32)
    for b in range(BL):
        for m in range(M):
            cstb[b * M + m, _CB_SEL32 + b] = 1.0
    cstb[0:H, _CB_ONES12] = 1.0
    cstb[0:128, _CB_ONES128C] = 1.0

    # [32,128] replicators, variant v covers bl-chunk rows 16v..16v+15,
    # tiled at every 32-partition base so lhsT/rhs partition bases match
    sab32 = np.zeros((32, _SAB_NCOL), np.float32)
    p = np.arange(128)
    srcA = (p // 64) * 8 + (p % 64) // 8
    srcB = (p // 64) * 8 + (p % 8)
    for v in range(2):
        sab32[16 * v + srcA, v * 128 + p] = 1.0
        sab32[16 * v + srcB, (2 + v) * 128 + p] = 1.0
    sab = np.tile(sab32, (4, 1))
    return whT, wtT, wbT, cstf, _bf16(cstb), _bf16(sab)


def _make_in_maps(inputs):
    seq = np.ascontiguousarray(np.asarray(inputs["sequence_output"], np.float32))
    att = np.ascontiguousarray(np.asarray(inputs["attention"], np.float32))
    ner = np.asarray(inputs["ner_tags"], np.float32)
    ep = np.asarray(inputs["entity_pos"]).astype(np.int64)
    pos = ep + OFFSET  # [B, 2, M]

    whT, wtT, wbT, cstf, cstb, sab = _weights_prep(
        np.asarray(inputs["W_head"]),
        np.asarray(inputs["b_head"]),
        np.asarray(inputs["W_tail"]),
        np.asarray(inputs["b_tail"]),
        np.asarray(inputs["W_bil"]),
        np.asarray(inputs["b_bil"]),
    )

    in_maps = []
    for k in range(NCORES):
        b0 = k * BL
        seq_k = _bf16(seq[b0 : b0 + BL].reshape(BL * C, D))
        att_k = att[b0 : b0 + BL].reshape(BL * H * C, C)

        midx = np.zeros((BL * M, 1), np.int32)
        for b in range(BL):
            midx[b * M : (b + 1) * M, 0] = b * C + pos[b0 + b, 0, :]

        aidx = np.zeros((M * H, 2 * BL), np.int32)
        mh_h = np.tile(np.arange(H), M)  # row p = m*H + h -> h
        mh_m = np.repeat(np.arange(M), H)  # -> m
        for b in range(BL):
            for e in range(2):
                aidx[:, b * 2 + e] = (b * H + mh_h) * C + pos[b0 + b, e, mh_m]

        nert = np.zeros((2 * (NER + 1), BL), np.float32)
        nert[0:NER] = ner[b0 : b0 + BL, 0, :].T
        nert[NER] = 1.0
        nert[NER + 1 : 2 * NER + 1] = ner[b0 : b0 + BL, 1, :].T
        nert[2 * NER + 1] = 1.0

        in_maps.append(
            {
                "seq": seq_k,
                "attn": np.ascontiguousarray(att_k),
                "midx": midx,
                "aidx": aidx,
                "nert": _bf16(nert),
                "whT": whT,
                "wtT": wtT,
                "wbT": wbT,
                "cstf": cstf,
                "cstb": cstb,
                "sab": sab,
            }
        )
    return in_maps


def _get_nc():
    if "nc" not in _CACHE:
        _CACHE["nc"] = _build_nc()
    return _CACHE["nc"]


def kernel(**inputs):
    global LAST_EXEC_NS, LAST_RESULTS
    nc = _get_nc()
    in_maps = _make_in_maps(inputs)
    trace = bool(int(os.environ.get("BASS_KERNEL_TRACE", "0")))
    try:
        res = run_bass_kernel_spmd(
            nc, in_maps, core_ids=list(range(NCORES)), trace=trace
        )
    except Exception:
        if not trace:
            raise
        # tracing infra unavailable in this environment -- run untraced
        res = run_bass_kernel_spmd(
            nc, in_maps, core_ids=list(range(NCORES)), trace=False
        )
    LAST_EXEC_NS = res.exec_time_ns
    LAST_RESULTS = res
    out = np.zeros((B, NCLS), np.float32)
    for k in range(NCORES):
        out[k * BL : (k + 1) * BL] = np.asarray(res.results[k]["outT"]).T
    return out
